# revision 26
# baseline (speedup 1.0000x reference)
"""Fused linear + cross-entropy loss (chunked logsumexp) on 8 NeuronCores.

Strategy: tensor-parallel over vocab. Each core holds a 4000-row shard of
head_weight, computes logits = h @ W_c^T for all 4096 tokens (fp8e4m3
DoubleRow matmuls by default; bf16 fallback when head_bias is nonzero),
and reduces sum(exp(logit)) per token on the ACT engine (exp with
accum_out; the pre-exp rescale for the fp8 weight scaling rides the
ACT's free scale operand). The target-logit term is a per-token dot
h[t] . W[label_t] computed on the DVE in bf16 from host-gathered rows
(data-parallel over tokens). The host does only glue: transpose/cast/
shard, the final log over 4096 values, and the weighted mean.

Startup/tail tuning: the first half's weight chunks and the first h
block are DMA'd in two k-halves each (2KB-per-partition runs keep the
rings at full rate), issued in exact consumption order across the sync
+ scalar queues, so the PE's first real matmul group waits on ~512KB
instead of ~2MB and the c-ordered early steps start as chunks land.
hsums is drained in two slices so the final output DMA is small.
"""

import numpy as np
import ml_dtypes

T = 4096
D = 1024
V = 32000
NCORES = 8
VSH = V // NCORES        # 4000 vocab rows per core
CPH = VSH // 2           # 2000 vocab cols per half
TT = T // 128            # 32 token tiles
TBC = 512                # tokens per resident ht block
NTB = T // TBC           # 8 ht col blocks
TLOC = T // NCORES       # 512 tokens per core for the target dot
JT = TLOC // 128         # 4 local token tiles

W_SCALE = 32.0           # fp8 path: W is scaled by this before casting
USE_FP8 = True
NT0 = 4                  # leading token tiles processed as mini+rest steps

_CACHE = {}


def _chunks(cols):
    """Split cols into matmul free-dim chunks (<=512, 16-aligned)."""
    out = []
    while cols > 0:
        c = min(cols, 512)
        out.append(c)
        cols -= c
    assert all(c % 16 == 0 for c in out)
    return out


def _ksplits(kt, n):
    """Split kt k-tiles into <=n contiguous pieces on matmul-group
    boundaries (even boundaries so fp8 DoubleRow reads stay in-piece)."""
    kh = -(-kt // n)
    if kh % 2 and kt % 2 == 0:
        kh += 1
    out = []
    k0 = 0
    while k0 < kt:
        k1 = min(k0 + kh, kt)
        out.append((k0, k1))
        k0 = k1
    return out


def _build(kt, mode, t=T, vsh=VSH, jt=JT, d=D, warm_n=36,
           do_compile=True):
    """Build+compile the SPMD Bass program.

    kt: number of 128-deep k tiles (8, or 9 when a nonzero head_bias is
        folded in as an extra contraction row).
    mode: "bf16" (plain matmuls) or "fp8dr" (fp8e4m3 DoubleRow, kt even).
    """
    import concourse.bass as bass
    import concourse.mybir as mybir
    import concourse.tile as tile
    from concourse import bacc

    f32 = mybir.dt.float32
    bf16 = mybir.dt.bfloat16
    fp8 = mybir.dt.float8e4
    AF = mybir.ActivationFunctionType
    ALU = mybir.AluOpType

    fp8dr = mode == "fp8dr"
    mdt = fp8 if fp8dr else bf16
    act_scale = (1.0 / W_SCALE) if fp8dr else 1.0
    if fp8dr:
        assert kt % 2 == 0
    nk = kt // 2 if fp8dr else kt   # matmul contraction steps

    tt = t // 128
    tb = min(TBC // 128, tt)   # token tiles per ht block
    ntb = tt // tb
    cph = vsh // 2
    CH = _chunks(cph)          # e.g. [512, 512, 512, 464]
    nch = len(CH)
    nsteps = 2 * tt
    nt0 = NT0                  # leading token tiles split mini+rest
    # extra accum cols: trailing halves of the last two steps at
    # nsteps/nsteps+1, minis after them
    ncols = nsteps + 2 + nt0

    nc = bacc.Bacc("TRN2", target_bir_lowering=False, debug=False)

    ht_d = nc.dram_tensor("ht", [ntb, 128, kt, tb * 128], mdt,
                          kind="ExternalInput")
    w_d = {}
    for half in range(2):
        for ci, w in enumerate(CH):
            w_d[half, ci] = nc.dram_tensor(
                f"w_{half}_{ci}", [128, kt, w], mdt, kind="ExternalInput"
            )
    hwg_d = nc.dram_tensor("hwg", [jt, 128, 2, d], bf16,
                           kind="ExternalInput")
    hsums_d = nc.dram_tensor("hsums", [128, ncols], f32,
                             kind="ExternalOutput")
    tgt_d = nc.dram_tensor("tgt", [128, jt], f32, kind="ExternalOutput")

    KS = _ksplits(kt, 2)       # DMA pieces for first-needed tiles

    with tile.TileContext(nc) as tc:
        with (
            tc.tile_pool(name="w", bufs=1) as wpool,
            tc.tile_pool(name="h", bufs=1) as hpool,
            tc.tile_pool(name="dot", bufs=1) as dpool,
            tc.tile_pool(name="stat", bufs=1) as spool,
            tc.tile_pool(name="sink", bufs=4) as kpool,
            tc.tile_pool(name="ps", bufs=2, space="PSUM") as ppool,
        ):
            wt = {}
            ht = [None] * ntb

            def load_w(half, ci):
                w = CH[ci]
                tl = wpool.tile([128, kt, w], mdt, tag=f"w{half}_{ci}")
                nc.sync.dma_start(tl[:], w_d[half, ci][:])
                wt[half, ci] = tl

            def load_h(b):
                tl = hpool.tile([128, kt, tb * 128], mdt, tag=f"h{b}")
                nc.sync.dma_start(tl[:], ht_d[b])
                ht[b] = tl

            # First-needed data first, in exact consumption order, in
            # two k-halves each (2KB per-partition runs keep the rings
            # at full rate) so the first matmul group waits on ~0.5MB
            # and each early c-ordered chunk starts as its half lands.
            # w halves ride the sync queue; h0 halves ride the scalar
            # queue (both trigger the same hardware rings, halving the
            # ~0.65us-per-issue serialization on the critical path).
            h0 = hpool.tile([128, kt, tb * 128], mdt, tag="h0")
            ht[0] = h0
            for ci in range(nch):
                w = CH[ci]
                tl = wpool.tile([128, kt, w], mdt, tag=f"w0_{ci}")
                wt[0, ci] = tl
                for k0, k1 in KS:
                    nc.sync.dma_start(
                        tl[:, k0:k1, :], w_d[0, ci][:, k0:k1, :]
                    )
                    if ci == 0:
                        nc.scalar.dma_start(
                            h0[:, k0:k1, :], ht_d[0, :, k0:k1, :]
                        )
            for b in range(1, ntb):
                load_h(b)
            for ci in range(nch):
                load_w(1, ci)

            # PE warmup during the DMA wait: junk matmuls from a memset
            # tile keep the HAM activity window busy so real matmuls run
            # at full clock. Writes the first ps slot; real groups clear
            # the bank with start=True before use.
            warm = kpool.tile([128, 256], mdt, tag="warm")
            nc.gpsimd.memset(warm[:], 0.0)
            ps_w = ppool.tile([128, nch, 512], f32, tag="ps")
            for _ in range(warm_n):
                nc.tensor.matmul(
                    ps_w[:, 0, 0:128], warm[:, 0:128], warm[:, 128:256],
                    start=True, stop=True,
                )

            # Target dot: tgt[p, j] = sum_d hwg[j,p,0,d] * hwg[j,p,1,d]
            tgt_sb = spool.tile([128, jt], f32, tag="tgt")
            for j in range(jt):
                hw = dpool.tile([128, 2, d], bf16, tag=f"hw{j}")
                nc.sync.dma_start(hw[:], hwg_d[j])
                dsink = kpool.tile([128, d], f32, tag="dsink")
                nc.vector.tensor_tensor(
                    dsink[:], hw[:, 0, :], hw[:, 1, :], ALU.mult
                )
                nc.vector.tensor_reduce(
                    tgt_sb[:, j:j + 1],
                    dsink[:],
                    axis=mybir.AxisListType.X,
                    op=ALU.add,
                )
            nc.sync.dma_start(tgt_d[:], tgt_sb[:])

            def mm(ps, hblk, mlo, half, ki, ci):
                rhs_t = wt[half, ci]
                w = CH[ci]
                if fp8dr:
                    nc.tensor.matmul(
                        ps[:, ci, 0:w],
                        hblk[:, 2 * ki:2 * ki + 2, mlo:mlo + 128],
                        rhs_t[:, 2 * ki:2 * ki + 2, :],
                        start=(ki == 0),
                        stop=(ki == nk - 1),
                        perf_mode=mybir.MatmulPerfMode.DoubleRow,
                    )
                else:
                    nc.tensor.matmul(
                        ps[:, ci, 0:w],
                        hblk[:, ki, mlo:mlo + 128],
                        rhs_t[:, ki, :],
                        start=(ki == 0),
                        stop=(ki == nk - 1),
                    )

            hsums = spool.tile([128, ncols], f32, tag="hsums")

            def act(ps, c0, c1, col):
                # One ACT over banks [c0, c1). Unwritten PSUM cols (the
                # tail of the last bank) read as zero, contributing
                # exp(0)=1 each; host subtracts them.
                esink = kpool.tile([128, nch * 512], bf16, tag="esink")
                nc.scalar.activation(
                    esink[:, c0 * 512:c1 * 512],
                    ps[:, c0:c1, :],
                    AF.Exp,
                    scale=act_scale,
                    accum_out=hsums[:, col:col + 1],
                )

            def step(half, t_i, order, c0=0, col=None):
                if col is None:
                    col = half * tt + t_i
                hblk = ht[t_i // tb]
                mlo = (t_i % tb) * 128
                ps = ppool.tile([128, nch, 512], f32, tag="ps")
                if order == "k":
                    for ki in range(nk):
                        for ci in range(c0, nch):
                            mm(ps, hblk, mlo, half, ki, ci)
                else:
                    for ci in range(c0, nch):
                        for ki in range(nk):
                            mm(ps, hblk, mlo, half, ki, ci)
                act(ps, c0, nch, col)

            # Leading token tiles: one "mini block" first — token tiles
            # 0..nt0-1 against chunk 0 only, one PSUM bank per tile,
            # k-outer so the first 8 matmuls need just the first
            # k-halves of w0_0 + h0 (~0.5MB, the first data to land).
            # Then chunks 1-3 per tile as their DMAs land. Keeps the PE
            # dense through the DMA-paced window so the HAM grant holds.
            w00t = wt[0, 0]

            def mini_mm(ps, bank, t_i, ki):
                mlo = (t_i % tb) * 128
                if fp8dr:
                    nc.tensor.matmul(
                        ps[:, bank, 0:CH[0]],
                        h0[:, 2 * ki:2 * ki + 2, mlo:mlo + 128],
                        w00t[:, 2 * ki:2 * ki + 2, :],
                        start=(ki == 0),
                        stop=(ki == nk - 1),
                        perf_mode=mybir.MatmulPerfMode.DoubleRow,
                    )
                else:
                    nc.tensor.matmul(
                        ps[:, bank, 0:CH[0]],
                        h0[:, ki, mlo:mlo + 128],
                        w00t[:, ki, :],
                        start=(ki == 0),
                        stop=(ki == nk - 1),
                    )

            # Two 2-tile mini allocations so the first pair's ACTs run
            # under the second pair's matmuls (no whole-block ACT chain
            # for the following rest-step to wait on).
            for pair in range(nt0 // 2):
                psm = ppool.tile([128, nch, 512], f32, tag="ps")
                for ki in range(nk):
                    for j in range(2):
                        mini_mm(psm, j, 2 * pair + j, ki)
                for j in range(2):
                    act(psm, j, j + 1, nsteps + 2 + 2 * pair + j)
            for t_i in range(nt0):
                step(0, t_i, "c", c0=1)
            for t_i in range(nt0, tt):
                step(0, t_i, "k")
            nc.sync.dma_start(hsums_d[:, 0:tt], hsums[:, 0:tt])
            nc.sync.dma_start(
                hsums_d[:, nsteps + 2:], hsums[:, nsteps + 2:]
            )
            for t_i in range(tt - 2):
                step(1, t_i, "k")
            nc.sync.dma_start(
                hsums_d[:, tt:nsteps - 2], hsums[:, tt:nsteps - 2]
            )
            # Last two steps split across both PSUM buffers so the ACT
            # pipeline drains with ~1us half-width ACTs and only one
            # such ACT follows the final matmul.
            for t_i in (tt - 2, tt - 1):
                hblk = ht[t_i // tb]
                mlo = (t_i % tb) * 128
                xcol = nsteps + (tt - 1 - t_i)
                psa = ppool.tile([128, nch, 512], f32, tag="ps")
                for ci in range(2):
                    for ki in range(nk):
                        mm(psa, hblk, mlo, 1, ki, ci)
                act(psa, 0, 2, tt + t_i)
                psb = ppool.tile([128, nch, 512], f32, tag="ps")
                for ci in range(2, nch):
                    for ki in range(nk):
                        mm(psb, hblk, mlo, 1, ki, ci)
                act(psb, 2, nch, xcol)
            nc.sync.dma_start(
                hsums_d[:, nsteps - 2:nsteps + 2],
                hsums[:, nsteps - 2:nsteps + 2],
            )


    if do_compile:
        nc.compile()
    return nc


def _get_nc(kt, mode, warm_n=36):
    key = (kt, mode, warm_n)
    if key not in _CACHE:
        _CACHE[key] = _build(kt, mode, warm_n=warm_n)
    return _CACHE[key]


def kernel(hidden_states, head_weight, head_bias, labels, loss_weight):
    from concourse.bass_utils import run_bass_kernel_spmd

    bf16 = ml_dtypes.bfloat16
    fp8 = ml_dtypes.float8_e4m3
    h = np.ascontiguousarray(np.asarray(hidden_states, dtype=np.float32))
    W = np.ascontiguousarray(np.asarray(head_weight, dtype=np.float32))
    b = np.asarray(head_bias, dtype=np.float32)
    lab = np.asarray(labels).astype(np.int64)
    lw = np.asarray(loss_weight, dtype=np.float32)

    use_bias = bool(np.any(b))
    mode = "fp8dr" if (USE_FP8 and not use_bias) else "bf16"
    mdt = fp8 if mode == "fp8dr" else bf16
    wscale = W_SCALE if mode == "fp8dr" else 1.0
    kt = 9 if use_bias else 8
    nc = _get_nc(kt, mode)
    CH = _chunks(CPH)
    nsteps = 2 * TT

    # hT[k, p, t] = h[t, k*128+p]; ht blocks [ntb, 128, kt, TBC].
    hT = np.zeros((kt, 128, T), dtype=np.float32)
    hT[:8] = np.ascontiguousarray(h.T).reshape(8, 128, T)
    if use_bias:
        hT[8, 0, :] = 1.0
    ht_blocks = np.ascontiguousarray(
        hT.reshape(kt, 128, NTB, TBC).transpose(2, 1, 0, 3).astype(mdt)
    )

    Wg = W[lab]                     # [T, D] gathered target rows
    tgt_bias = b[lab]               # [T]

    in_maps = []
    for c in range(NCORES):
        Wc = np.ascontiguousarray(W[c * VSH:(c + 1) * VSH].T) * wscale
        # wT[k, p, v] = Wc.T[k*128+p, v] (scaled)
        wT = np.zeros((kt, 128, VSH), dtype=np.float32)
        wT[:8] = Wc.reshape(8, 128, VSH)
        if use_bias:
            wT[8, 0, :] = b[c * VSH:(c + 1) * VSH]
        m = {}
        off = 0
        for half in range(2):
            for ci, w in enumerate(CH):
                blk = wT[:, :, off:off + w].transpose(1, 0, 2).astype(mdt)
                m[f"w_{half}_{ci}"] = np.ascontiguousarray(blk)
                off += w
        m["ht"] = ht_blocks
        hrow = h[c * TLOC:(c + 1) * TLOC].reshape(JT, 128, D)
        wgrow = Wg[c * TLOC:(c + 1) * TLOC].reshape(JT, 128, D)
        m["hwg"] = np.ascontiguousarray(
            np.stack([hrow, wgrow], axis=2).astype(bf16)
        )
        in_maps.append(m)

    # Tile's scheduler is nondeterministic across builds and has a rare
    # dependency-emission bug: a bad roll yields a NEFF whose outputs are
    # corrupt (dropped accum slots / garbage operands). Validate against
    # hard invariants and an exact host check of the target dots; on
    # failure, rebuild (fresh schedule roll) and rerun.
    pad = len(CH) * 512 - CPH          # zero-region cols per step
    f32 = np.float32

    # Exact host reference for every target dot (same bf16 operands).
    tgt_ref = np.stack([
        (im["hwg"][:, :, 0].astype(f32) * im["hwg"][:, :, 1].astype(f32))
        .sum(axis=2).reshape(TLOC)
        for im in in_maps
    ])                                                      # [8, TLOC]

    # One probe token per token tile, per core: replicates the device's
    # quantized math exactly (same casts) so every accum slot is checked.
    probe_p = (np.arange(TT) * 37) % 128
    probe_tok = np.arange(TT) * 128 + probe_p
    hq = h.astype(mdt).astype(f32)[probe_tok]               # [TT, D]
    if use_bias:
        hq = np.concatenate([hq, np.ones((TT, 1), f32)], axis=1)
    probe_ref = np.empty((NCORES, TT), f32)
    for c in range(NCORES):
        Wc = np.ascontiguousarray(W[c * VSH:(c + 1) * VSH]) * wscale
        Wq = Wc.astype(mdt).astype(f32)                     # [VSH, D]
        if use_bias:
            bq = b[c * VSH:(c + 1) * VSH].astype(mdt).astype(f32)
            Wq = np.concatenate([Wq, bq[:, None]], axis=1)
        lg = (hq @ Wq.T) / wscale
        probe_ref[c] = np.exp(lg).sum(axis=1)

    for attempt in range(4):
        res = run_bass_kernel_spmd(nc, in_maps, core_ids=list(range(NCORES)))

        # hsums[c][p, half*TT+t] are partial sums of exp(logit) over half
        # of core c's vocab shard for token t*128+p (+pad zero-cols).
        # Extra cols: [nsteps]/[nsteps+1] = trailing banks of the last
        # two steps, [nsteps+2+t] = chunk-0 minis of the leading token
        # tiles; fold them in.
        Sfull = np.stack([r["hsums"] for r in res.results])  # [8,128,ncols]
        Sraw = np.ascontiguousarray(Sfull[:, :, :nsteps])
        Sraw[:, :, nsteps - 1] += Sfull[:, :, nsteps]
        Sraw[:, :, nsteps - 2] += Sfull[:, :, nsteps + 1]
        Sraw[:, :, :NT0] += Sfull[:, :, nsteps + 2:]
        G = np.stack([r["tgt"] for r in res.results])       # [8, 128, JT]
        err_state = np.seterr(over="ignore", invalid="ignore")
        dev_probe = (
            Sraw[:, probe_p, np.arange(TT)]
            + Sraw[:, probe_p, TT + np.arange(TT)]
            - 2.0 * pad
        )                                                   # [8, TT]
        g_dev = G.transpose(0, 2, 1).reshape(NCORES, TLOC)
        ok = (
            np.isfinite(Sfull).all()
            and np.isfinite(G).all()
            and (Sraw > pad).all()
            and np.allclose(g_dev, tgt_ref, rtol=2e-2, atol=1e-2)
            and np.allclose(dev_probe, probe_ref, rtol=5e-2, atol=1.0)
        )
        np.seterr(**err_state)
        if ok:
            break
        nc = _get_nc(kt, mode, warm_n=36 + 2 * (attempt + 1))
    if not ok:
        # Every compile rolled a bad schedule: compute on host (slow but
        # exact) rather than return a corrupt result.
        logits = h @ W.T + b
        mx = logits.max(axis=1, keepdims=True)
        logz = np.log(
            np.exp((logits - mx).astype(np.float64)).sum(axis=1)
        ) + mx[:, 0]
        nll = logz - logits[np.arange(T), lab]
        lw64 = lw.astype(np.float64)
        return np.float32((lw64 * nll).sum() / lw64.sum())

    S = Sraw.reshape(NCORES, 128, 2, TT).sum(axis=2)        # [8,128,TT]
    sumexp = S.transpose(0, 2, 1).reshape(NCORES, T).astype(np.float64)
    sumexp -= 2.0 * pad
    logz = np.log(sumexp.sum(axis=0))                       # [T]

    tgt = G.transpose(0, 2, 1).reshape(T) + tgt_bias        # [T]

    nll = logz - tgt
    lw64 = lw.astype(np.float64)
    loss = (lw64 * nll).sum() / lw64.sum()
    return np.float32(loss)


# revision 27
# speedup vs baseline: 1.0073x; 1.0073x over previous
"""Fused linear + cross-entropy loss (chunked logsumexp) on 8 NeuronCores.

Strategy: tensor-parallel over vocab. Each core holds a 4000-row shard of
head_weight, computes logits = h @ W_c^T for all 4096 tokens (fp8e4m3
DoubleRow matmuls by default; bf16 fallback when head_bias is nonzero),
and reduces sum(exp(logit)) per token on the ACT engine (exp with
accum_out; the pre-exp rescale for the fp8 weight scaling rides the
ACT's free scale operand). The target-logit term is a per-token dot
h[t] . W[label_t] computed on the DVE in bf16 from host-gathered rows
(data-parallel over tokens). The host does only glue: transpose/cast/
shard, the final log over 4096 values, and the weighted mean.

Startup/tail tuning: the first half's weight chunks and the first h
block are DMA'd in two k-halves each (2KB-per-partition runs keep the
rings at full rate), issued in exact consumption order across the sync
+ scalar queues, so the PE's first real matmul group waits on ~512KB
instead of ~2MB and the c-ordered early steps start as chunks land.
hsums is drained in two slices so the final output DMA is small.
"""

import numpy as np
import ml_dtypes

T = 4096
D = 1024
V = 32000
NCORES = 8
VSH = V // NCORES        # 4000 vocab rows per core
CPH = VSH // 2           # 2000 vocab cols per half
TT = T // 128            # 32 token tiles
TBC = 512                # tokens per resident ht block
NTB = T // TBC           # 8 ht col blocks
TLOC = T // NCORES       # 512 tokens per core for the target dot
JT = TLOC // 128         # 4 local token tiles

W_SCALE = 32.0           # fp8 path: W is scaled by this before casting
USE_FP8 = True
NT0 = 4                  # leading token tiles processed as mini+rest steps

_CACHE = {}


def _chunks(cols):
    """Split cols into matmul free-dim chunks (<=512, 16-aligned)."""
    out = []
    while cols > 0:
        c = min(cols, 512)
        out.append(c)
        cols -= c
    assert all(c % 16 == 0 for c in out)
    return out


def _ksplits(kt, n):
    """Split kt k-tiles into <=n contiguous pieces on matmul-group
    boundaries (even boundaries so fp8 DoubleRow reads stay in-piece)."""
    kh = -(-kt // n)
    if kh % 2 and kt % 2 == 0:
        kh += 1
    out = []
    k0 = 0
    while k0 < kt:
        k1 = min(k0 + kh, kt)
        out.append((k0, k1))
        k0 = k1
    return out


def _build(kt, mode, t=T, vsh=VSH, jt=JT, d=D, warm_n=44,
           do_compile=True):
    """Build+compile the SPMD Bass program.

    kt: number of 128-deep k tiles (8, or 9 when a nonzero head_bias is
        folded in as an extra contraction row).
    mode: "bf16" (plain matmuls) or "fp8dr" (fp8e4m3 DoubleRow, kt even).
    """
    import concourse.bass as bass
    import concourse.mybir as mybir
    import concourse.tile as tile
    from concourse import bacc

    f32 = mybir.dt.float32
    bf16 = mybir.dt.bfloat16
    fp8 = mybir.dt.float8e4
    AF = mybir.ActivationFunctionType
    ALU = mybir.AluOpType

    fp8dr = mode == "fp8dr"
    mdt = fp8 if fp8dr else bf16
    act_scale = (1.0 / W_SCALE) if fp8dr else 1.0
    if fp8dr:
        assert kt % 2 == 0
    nk = kt // 2 if fp8dr else kt   # matmul contraction steps

    tt = t // 128
    tb = min(TBC // 128, tt)   # token tiles per ht block
    ntb = tt // tb
    cph = vsh // 2
    CH = _chunks(cph)          # e.g. [512, 512, 512, 464]
    nch = len(CH)
    nsteps = 2 * tt
    nt0 = NT0                  # leading token tiles split mini+rest
    # extra accum cols: trailing halves of the last two steps at
    # nsteps/nsteps+1, minis after them
    ncols = nsteps + 2 + nt0

    nc = bacc.Bacc("TRN2", target_bir_lowering=False, debug=False)

    ht_d = nc.dram_tensor("ht", [ntb, 128, kt, tb * 128], mdt,
                          kind="ExternalInput")
    w_d = {}
    for half in range(2):
        for ci, w in enumerate(CH):
            w_d[half, ci] = nc.dram_tensor(
                f"w_{half}_{ci}", [128, kt, w], mdt, kind="ExternalInput"
            )
    hwg_d = nc.dram_tensor("hwg", [jt, 128, 2, d], bf16,
                           kind="ExternalInput")
    hsums_d = nc.dram_tensor("hsums", [128, ncols], f32,
                             kind="ExternalOutput")
    tgt_d = nc.dram_tensor("tgt", [128, jt], f32, kind="ExternalOutput")

    KS = _ksplits(kt, 2)       # DMA pieces for first-needed tiles

    with tile.TileContext(nc) as tc:
        with (
            tc.tile_pool(name="w", bufs=1) as wpool,
            tc.tile_pool(name="h", bufs=1) as hpool,
            tc.tile_pool(name="dot", bufs=1) as dpool,
            tc.tile_pool(name="stat", bufs=1) as spool,
            tc.tile_pool(name="sink", bufs=4) as kpool,
            tc.tile_pool(name="ps", bufs=2, space="PSUM") as ppool,
        ):
            wt = {}
            ht = [None] * ntb

            def load_w(half, ci):
                w = CH[ci]
                tl = wpool.tile([128, kt, w], mdt, tag=f"w{half}_{ci}")
                nc.sync.dma_start(tl[:], w_d[half, ci][:])
                wt[half, ci] = tl

            def load_h(b):
                tl = hpool.tile([128, kt, tb * 128], mdt, tag=f"h{b}")
                nc.sync.dma_start(tl[:], ht_d[b])
                ht[b] = tl

            # First-needed data first, in exact consumption order, in
            # two k-halves each (2KB per-partition runs keep the rings
            # at full rate) so the first matmul group waits on ~0.5MB
            # and each early c-ordered chunk starts as its half lands.
            # w halves ride the sync queue; h0 halves ride the scalar
            # queue (both trigger the same hardware rings, halving the
            # ~0.65us-per-issue serialization on the critical path).
            h0 = hpool.tile([128, kt, tb * 128], mdt, tag="h0")
            ht[0] = h0
            for ci in range(nch):
                w = CH[ci]
                tl = wpool.tile([128, kt, w], mdt, tag=f"w0_{ci}")
                wt[0, ci] = tl
                for k0, k1 in KS:
                    nc.sync.dma_start(
                        tl[:, k0:k1, :], w_d[0, ci][:, k0:k1, :]
                    )
                    if ci == 0:
                        nc.scalar.dma_start(
                            h0[:, k0:k1, :], ht_d[0, :, k0:k1, :]
                        )
            for b in range(1, ntb):
                load_h(b)
            for ci in range(nch):
                load_w(1, ci)

            # PE warmup during the DMA wait: junk matmuls from a memset
            # tile keep the HAM activity window busy so real matmuls run
            # at full clock. Writes the first ps slot; real groups clear
            # the bank with start=True before use.
            warm = kpool.tile([128, 256], mdt, tag="warm")
            nc.gpsimd.memset(warm[:], 0.0)
            ps_w = ppool.tile([128, nch, 512], f32, tag="ps")
            for _ in range(warm_n):
                nc.tensor.matmul(
                    ps_w[:, 0, 0:128], warm[:, 0:128], warm[:, 128:256],
                    start=True, stop=True,
                )

            # Target dot: tgt[p, j] = sum_d hwg[j,p,0,d] * hwg[j,p,1,d]
            tgt_sb = spool.tile([128, jt], f32, tag="tgt")
            for j in range(jt):
                hw = dpool.tile([128, 2, d], bf16, tag=f"hw{j}")
                nc.sync.dma_start(hw[:], hwg_d[j])
                dsink = kpool.tile([128, d], f32, tag="dsink")
                nc.vector.tensor_tensor(
                    dsink[:], hw[:, 0, :], hw[:, 1, :], ALU.mult
                )
                nc.vector.tensor_reduce(
                    tgt_sb[:, j:j + 1],
                    dsink[:],
                    axis=mybir.AxisListType.X,
                    op=ALU.add,
                )
            nc.sync.dma_start(tgt_d[:], tgt_sb[:])

            def mm(ps, hblk, mlo, half, ki, ci):
                rhs_t = wt[half, ci]
                w = CH[ci]
                if fp8dr:
                    nc.tensor.matmul(
                        ps[:, ci, 0:w],
                        hblk[:, 2 * ki:2 * ki + 2, mlo:mlo + 128],
                        rhs_t[:, 2 * ki:2 * ki + 2, :],
                        start=(ki == 0),
                        stop=(ki == nk - 1),
                        perf_mode=mybir.MatmulPerfMode.DoubleRow,
                    )
                else:
                    nc.tensor.matmul(
                        ps[:, ci, 0:w],
                        hblk[:, ki, mlo:mlo + 128],
                        rhs_t[:, ki, :],
                        start=(ki == 0),
                        stop=(ki == nk - 1),
                    )

            hsums = spool.tile([128, ncols], f32, tag="hsums")

            def act(ps, c0, c1, col):
                # One ACT over banks [c0, c1). Unwritten PSUM cols (the
                # tail of the last bank) read as zero, contributing
                # exp(0)=1 each; host subtracts them.
                esink = kpool.tile([128, nch * 512], bf16, tag="esink")
                nc.scalar.activation(
                    esink[:, c0 * 512:c1 * 512],
                    ps[:, c0:c1, :],
                    AF.Exp,
                    scale=act_scale,
                    accum_out=hsums[:, col:col + 1],
                )

            def step(half, t_i, order, c0=0, col=None):
                if col is None:
                    col = half * tt + t_i
                hblk = ht[t_i // tb]
                mlo = (t_i % tb) * 128
                ps = ppool.tile([128, nch, 512], f32, tag="ps")
                if order == "k":
                    for ki in range(nk):
                        for ci in range(c0, nch):
                            mm(ps, hblk, mlo, half, ki, ci)
                else:
                    for ci in range(c0, nch):
                        for ki in range(nk):
                            mm(ps, hblk, mlo, half, ki, ci)
                act(ps, c0, nch, col)

            # Leading token tiles: one "mini block" first — token tiles
            # 0..nt0-1 against chunk 0 only, one PSUM bank per tile,
            # k-outer so the first 8 matmuls need just the first
            # k-halves of w0_0 + h0 (~0.5MB, the first data to land).
            # Then chunks 1-3 per tile as their DMAs land. Keeps the PE
            # dense through the DMA-paced window so the HAM grant holds.
            w00t = wt[0, 0]

            def mini_mm(ps, bank, t_i, ki):
                mlo = (t_i % tb) * 128
                if fp8dr:
                    nc.tensor.matmul(
                        ps[:, bank, 0:CH[0]],
                        h0[:, 2 * ki:2 * ki + 2, mlo:mlo + 128],
                        w00t[:, 2 * ki:2 * ki + 2, :],
                        start=(ki == 0),
                        stop=(ki == nk - 1),
                        perf_mode=mybir.MatmulPerfMode.DoubleRow,
                    )
                else:
                    nc.tensor.matmul(
                        ps[:, bank, 0:CH[0]],
                        h0[:, ki, mlo:mlo + 128],
                        w00t[:, ki, :],
                        start=(ki == 0),
                        stop=(ki == nk - 1),
                    )

            # Two 2-tile mini allocations so the first pair's ACTs run
            # under the second pair's matmuls (no whole-block ACT chain
            # for the following rest-step to wait on).
            for pair in range(nt0 // 2):
                psm = ppool.tile([128, nch, 512], f32, tag="ps")
                for ki in range(nk):
                    for j in range(2):
                        mini_mm(psm, j, 2 * pair + j, ki)
                for j in range(2):
                    act(psm, j, j + 1, nsteps + 2 + 2 * pair + j)
            for t_i in range(nt0):
                step(0, t_i, "c", c0=1)
            for t_i in range(nt0, tt):
                step(0, t_i, "k")
            nc.sync.dma_start(hsums_d[:, 0:tt], hsums[:, 0:tt])
            nc.sync.dma_start(
                hsums_d[:, nsteps + 2:], hsums[:, nsteps + 2:]
            )
            for t_i in range(tt - 2):
                step(1, t_i, "k")
            nc.sync.dma_start(
                hsums_d[:, tt:nsteps - 2], hsums[:, tt:nsteps - 2]
            )
            # Last two steps split across both PSUM buffers so the ACT
            # pipeline drains with ~1us half-width ACTs and only one
            # such ACT follows the final matmul.
            for t_i in (tt - 2, tt - 1):
                hblk = ht[t_i // tb]
                mlo = (t_i % tb) * 128
                xcol = nsteps + (tt - 1 - t_i)
                psa = ppool.tile([128, nch, 512], f32, tag="ps")
                for ci in range(2):
                    for ki in range(nk):
                        mm(psa, hblk, mlo, 1, ki, ci)
                act(psa, 0, 2, tt + t_i)
                psb = ppool.tile([128, nch, 512], f32, tag="ps")
                for ci in range(2, nch):
                    for ki in range(nk):
                        mm(psb, hblk, mlo, 1, ki, ci)
                act(psb, 2, nch, xcol)
            nc.sync.dma_start(
                hsums_d[:, nsteps - 2:nsteps + 2],
                hsums[:, nsteps - 2:nsteps + 2],
            )


    if do_compile:
        nc.compile()
    return nc


def _get_nc(kt, mode, warm_n=44):
    key = (kt, mode, warm_n)
    if key not in _CACHE:
        _CACHE[key] = _build(kt, mode, warm_n=warm_n)
    return _CACHE[key]


def kernel(hidden_states, head_weight, head_bias, labels, loss_weight):
    from concourse.bass_utils import run_bass_kernel_spmd

    bf16 = ml_dtypes.bfloat16
    fp8 = ml_dtypes.float8_e4m3
    h = np.ascontiguousarray(np.asarray(hidden_states, dtype=np.float32))
    W = np.ascontiguousarray(np.asarray(head_weight, dtype=np.float32))
    b = np.asarray(head_bias, dtype=np.float32)
    lab = np.asarray(labels).astype(np.int64)
    lw = np.asarray(loss_weight, dtype=np.float32)

    use_bias = bool(np.any(b))
    mode = "fp8dr" if (USE_FP8 and not use_bias) else "bf16"
    mdt = fp8 if mode == "fp8dr" else bf16
    wscale = W_SCALE if mode == "fp8dr" else 1.0
    kt = 9 if use_bias else 8
    nc = _get_nc(kt, mode)
    CH = _chunks(CPH)
    nsteps = 2 * TT

    # hT[k, p, t] = h[t, k*128+p]; ht blocks [ntb, 128, kt, TBC].
    hT = np.zeros((kt, 128, T), dtype=np.float32)
    hT[:8] = np.ascontiguousarray(h.T).reshape(8, 128, T)
    if use_bias:
        hT[8, 0, :] = 1.0
    ht_blocks = np.ascontiguousarray(
        hT.reshape(kt, 128, NTB, TBC).transpose(2, 1, 0, 3).astype(mdt)
    )

    Wg = W[lab]                     # [T, D] gathered target rows
    tgt_bias = b[lab]               # [T]

    in_maps = []
    for c in range(NCORES):
        Wc = np.ascontiguousarray(W[c * VSH:(c + 1) * VSH].T) * wscale
        # wT[k, p, v] = Wc.T[k*128+p, v] (scaled)
        wT = np.zeros((kt, 128, VSH), dtype=np.float32)
        wT[:8] = Wc.reshape(8, 128, VSH)
        if use_bias:
            wT[8, 0, :] = b[c * VSH:(c + 1) * VSH]
        m = {}
        off = 0
        for half in range(2):
            for ci, w in enumerate(CH):
                blk = wT[:, :, off:off + w].transpose(1, 0, 2).astype(mdt)
                m[f"w_{half}_{ci}"] = np.ascontiguousarray(blk)
                off += w
        m["ht"] = ht_blocks
        hrow = h[c * TLOC:(c + 1) * TLOC].reshape(JT, 128, D)
        wgrow = Wg[c * TLOC:(c + 1) * TLOC].reshape(JT, 128, D)
        m["hwg"] = np.ascontiguousarray(
            np.stack([hrow, wgrow], axis=2).astype(bf16)
        )
        in_maps.append(m)

    # Tile's scheduler is nondeterministic across builds and has a rare
    # dependency-emission bug: a bad roll yields a NEFF whose outputs are
    # corrupt (dropped accum slots / garbage operands). Validate against
    # hard invariants and an exact host check of the target dots; on
    # failure, rebuild (fresh schedule roll) and rerun.
    pad = len(CH) * 512 - CPH          # zero-region cols per step
    f32 = np.float32

    # Exact host reference for every target dot (same bf16 operands).
    tgt_ref = np.stack([
        (im["hwg"][:, :, 0].astype(f32) * im["hwg"][:, :, 1].astype(f32))
        .sum(axis=2).reshape(TLOC)
        for im in in_maps
    ])                                                      # [8, TLOC]

    # One probe token per token tile, per core: replicates the device's
    # quantized math exactly (same casts) so every accum slot is checked.
    probe_p = (np.arange(TT) * 37) % 128
    probe_tok = np.arange(TT) * 128 + probe_p
    hq = h.astype(mdt).astype(f32)[probe_tok]               # [TT, D]
    if use_bias:
        hq = np.concatenate([hq, np.ones((TT, 1), f32)], axis=1)
    probe_ref = np.empty((NCORES, TT), f32)
    for c in range(NCORES):
        Wc = np.ascontiguousarray(W[c * VSH:(c + 1) * VSH]) * wscale
        Wq = Wc.astype(mdt).astype(f32)                     # [VSH, D]
        if use_bias:
            bq = b[c * VSH:(c + 1) * VSH].astype(mdt).astype(f32)
            Wq = np.concatenate([Wq, bq[:, None]], axis=1)
        lg = (hq @ Wq.T) / wscale
        probe_ref[c] = np.exp(lg).sum(axis=1)

    for attempt in range(4):
        res = run_bass_kernel_spmd(nc, in_maps, core_ids=list(range(NCORES)))

        # hsums[c][p, half*TT+t] are partial sums of exp(logit) over half
        # of core c's vocab shard for token t*128+p (+pad zero-cols).
        # Extra cols: [nsteps]/[nsteps+1] = trailing banks of the last
        # two steps, [nsteps+2+t] = chunk-0 minis of the leading token
        # tiles; fold them in.
        Sfull = np.stack([r["hsums"] for r in res.results])  # [8,128,ncols]
        Sraw = np.ascontiguousarray(Sfull[:, :, :nsteps])
        Sraw[:, :, nsteps - 1] += Sfull[:, :, nsteps]
        Sraw[:, :, nsteps - 2] += Sfull[:, :, nsteps + 1]
        Sraw[:, :, :NT0] += Sfull[:, :, nsteps + 2:]
        G = np.stack([r["tgt"] for r in res.results])       # [8, 128, JT]
        err_state = np.seterr(over="ignore", invalid="ignore")
        dev_probe = (
            Sraw[:, probe_p, np.arange(TT)]
            + Sraw[:, probe_p, TT + np.arange(TT)]
            - 2.0 * pad
        )                                                   # [8, TT]
        g_dev = G.transpose(0, 2, 1).reshape(NCORES, TLOC)
        ok = (
            np.isfinite(Sfull).all()
            and np.isfinite(G).all()
            and (Sraw > pad).all()
            and np.allclose(g_dev, tgt_ref, rtol=2e-2, atol=1e-2)
            and np.allclose(dev_probe, probe_ref, rtol=5e-2, atol=1.0)
        )
        np.seterr(**err_state)
        if ok:
            break
        nc = _get_nc(kt, mode, warm_n=44 + 2 * (attempt + 1))
    if not ok:
        # Every compile rolled a bad schedule: compute on host (slow but
        # exact) rather than return a corrupt result.
        logits = h @ W.T + b
        mx = logits.max(axis=1, keepdims=True)
        logz = np.log(
            np.exp((logits - mx).astype(np.float64)).sum(axis=1)
        ) + mx[:, 0]
        nll = logz - logits[np.arange(T), lab]
        lw64 = lw.astype(np.float64)
        return np.float32((lw64 * nll).sum() / lw64.sum())

    S = Sraw.reshape(NCORES, 128, 2, TT).sum(axis=2)        # [8,128,TT]
    sumexp = S.transpose(0, 2, 1).reshape(NCORES, T).astype(np.float64)
    sumexp -= 2.0 * pad
    logz = np.log(sumexp.sum(axis=0))                       # [T]

    tgt = G.transpose(0, 2, 1).reshape(T) + tgt_bias        # [T]

    nll = logz - tgt
    lw64 = lw.astype(np.float64)
    loss = (lw64 * nll).sum() / lw64.sum()
    return np.float32(loss)


# revision 28
# speedup vs baseline: 1.0086x; 1.0013x over previous
"""Fused linear + cross-entropy loss (chunked logsumexp) on 8 NeuronCores.

Strategy: tensor-parallel over vocab. Each core holds a 4000-row shard of
head_weight, computes logits = h @ W_c^T for all 4096 tokens (fp8e4m3
DoubleRow matmuls by default; bf16 fallback when head_bias is nonzero),
and reduces sum(exp(logit)) per token on the ACT engine (exp with
accum_out; the pre-exp rescale for the fp8 weight scaling rides the
ACT's free scale operand). The target-logit term is a per-token dot
h[t] . W[label_t] computed on the DVE in bf16 from host-gathered rows
(data-parallel over tokens). The host does only glue: transpose/cast/
shard, the final log over 4096 values, and the weighted mean.

Startup/tail tuning: the first half's weight chunks and the first h
block are DMA'd in two k-halves each (2KB-per-partition runs keep the
rings at full rate), issued in exact consumption order across the sync
+ scalar queues, so the PE's first real matmul group waits on ~512KB
instead of ~2MB and the c-ordered early steps start as chunks land.
hsums is drained in two slices so the final output DMA is small.
"""

import numpy as np
import ml_dtypes

T = 4096
D = 1024
V = 32000
NCORES = 8
VSH = V // NCORES        # 4000 vocab rows per core
CPH = VSH // 2           # 2000 vocab cols per half
TT = T // 128            # 32 token tiles
TBC = 512                # tokens per resident ht block
NTB = T // TBC           # 8 ht col blocks
TLOC = T // NCORES       # 512 tokens per core for the target dot
JT = TLOC // 128         # 4 local token tiles

W_SCALE = 32.0           # fp8 path: W is scaled by this before casting
USE_FP8 = True
NT0 = 4                  # leading token tiles processed as mini+rest steps

_CACHE = {}


def _chunks(cols):
    """Split cols into matmul free-dim chunks (<=512, 16-aligned)."""
    out = []
    while cols > 0:
        c = min(cols, 512)
        out.append(c)
        cols -= c
    assert all(c % 16 == 0 for c in out)
    return out


def _ksplits(kt, n):
    """Split kt k-tiles into <=n contiguous pieces on matmul-group
    boundaries (even boundaries so fp8 DoubleRow reads stay in-piece)."""
    kh = -(-kt // n)
    if kh % 2 and kt % 2 == 0:
        kh += 1
    out = []
    k0 = 0
    while k0 < kt:
        k1 = min(k0 + kh, kt)
        out.append((k0, k1))
        k0 = k1
    return out


def _build(kt, mode, t=T, vsh=VSH, jt=JT, d=D, warm_n=48,
           do_compile=True):
    """Build+compile the SPMD Bass program.

    kt: number of 128-deep k tiles (8, or 9 when a nonzero head_bias is
        folded in as an extra contraction row).
    mode: "bf16" (plain matmuls) or "fp8dr" (fp8e4m3 DoubleRow, kt even).
    """
    import concourse.bass as bass
    import concourse.mybir as mybir
    import concourse.tile as tile
    from concourse import bacc

    f32 = mybir.dt.float32
    bf16 = mybir.dt.bfloat16
    fp8 = mybir.dt.float8e4
    AF = mybir.ActivationFunctionType
    ALU = mybir.AluOpType

    fp8dr = mode == "fp8dr"
    mdt = fp8 if fp8dr else bf16
    act_scale = (1.0 / W_SCALE) if fp8dr else 1.0
    if fp8dr:
        assert kt % 2 == 0
    nk = kt // 2 if fp8dr else kt   # matmul contraction steps

    tt = t // 128
    tb = min(TBC // 128, tt)   # token tiles per ht block
    ntb = tt // tb
    cph = vsh // 2
    CH = _chunks(cph)          # e.g. [512, 512, 512, 464]
    nch = len(CH)
    nsteps = 2 * tt
    nt0 = NT0                  # leading token tiles split mini+rest
    # extra accum cols: trailing halves of the last two steps at
    # nsteps/nsteps+1, minis after them
    ncols = nsteps + 2 + nt0

    nc = bacc.Bacc("TRN2", target_bir_lowering=False, debug=False)

    ht_d = nc.dram_tensor("ht", [ntb, 128, kt, tb * 128], mdt,
                          kind="ExternalInput")
    w_d = {}
    for half in range(2):
        for ci, w in enumerate(CH):
            w_d[half, ci] = nc.dram_tensor(
                f"w_{half}_{ci}", [128, kt, w], mdt, kind="ExternalInput"
            )
    hwg_d = nc.dram_tensor("hwg", [jt, 128, 2, d], bf16,
                           kind="ExternalInput")
    hsums_d = nc.dram_tensor("hsums", [128, ncols], f32,
                             kind="ExternalOutput")
    tgt_d = nc.dram_tensor("tgt", [128, jt], f32, kind="ExternalOutput")

    KS = _ksplits(kt, 2)       # DMA pieces for first-needed tiles

    with tile.TileContext(nc) as tc:
        with (
            tc.tile_pool(name="w", bufs=1) as wpool,
            tc.tile_pool(name="h", bufs=1) as hpool,
            tc.tile_pool(name="dot", bufs=1) as dpool,
            tc.tile_pool(name="stat", bufs=1) as spool,
            tc.tile_pool(name="sink", bufs=4) as kpool,
            tc.tile_pool(name="ps", bufs=2, space="PSUM") as ppool,
        ):
            wt = {}
            ht = [None] * ntb

            def load_w(half, ci):
                w = CH[ci]
                tl = wpool.tile([128, kt, w], mdt, tag=f"w{half}_{ci}")
                nc.sync.dma_start(tl[:], w_d[half, ci][:])
                wt[half, ci] = tl

            def load_h(b):
                tl = hpool.tile([128, kt, tb * 128], mdt, tag=f"h{b}")
                nc.sync.dma_start(tl[:], ht_d[b])
                ht[b] = tl

            # First-needed data first, in exact consumption order, in
            # two k-halves each (2KB per-partition runs keep the rings
            # at full rate) so the first matmul group waits on ~0.5MB
            # and each early c-ordered chunk starts as its half lands.
            # w halves ride the sync queue; h0 halves ride the scalar
            # queue (both trigger the same hardware rings, halving the
            # ~0.65us-per-issue serialization on the critical path).
            h0 = hpool.tile([128, kt, tb * 128], mdt, tag="h0")
            ht[0] = h0
            for ci in range(nch):
                w = CH[ci]
                tl = wpool.tile([128, kt, w], mdt, tag=f"w0_{ci}")
                wt[0, ci] = tl
                for k0, k1 in KS:
                    nc.sync.dma_start(
                        tl[:, k0:k1, :], w_d[0, ci][:, k0:k1, :]
                    )
                    if ci == 0:
                        nc.scalar.dma_start(
                            h0[:, k0:k1, :], ht_d[0, :, k0:k1, :]
                        )
            for b in range(1, ntb):
                load_h(b)
            for ci in range(nch):
                load_w(1, ci)

            # PE warmup during the DMA wait: junk matmuls from a memset
            # tile keep the HAM activity window busy so real matmuls run
            # at full clock. Writes the first ps slot; real groups clear
            # the bank with start=True before use.
            warm = kpool.tile([128, 256], mdt, tag="warm")
            nc.gpsimd.memset(warm[:], 0.0)
            ps_w = ppool.tile([128, nch, 512], f32, tag="ps")
            for _ in range(warm_n):
                nc.tensor.matmul(
                    ps_w[:, 0, 0:128], warm[:, 0:128], warm[:, 128:256],
                    start=True, stop=True,
                )

            # Target dot: tgt[p, j] = sum_d hwg[j,p,0,d] * hwg[j,p,1,d]
            tgt_sb = spool.tile([128, jt], f32, tag="tgt")
            for j in range(jt):
                hw = dpool.tile([128, 2, d], bf16, tag=f"hw{j}")
                nc.sync.dma_start(hw[:], hwg_d[j])
                dsink = kpool.tile([128, d], f32, tag="dsink")
                nc.vector.tensor_tensor(
                    dsink[:], hw[:, 0, :], hw[:, 1, :], ALU.mult
                )
                nc.vector.tensor_reduce(
                    tgt_sb[:, j:j + 1],
                    dsink[:],
                    axis=mybir.AxisListType.X,
                    op=ALU.add,
                )
            nc.sync.dma_start(tgt_d[:], tgt_sb[:])

            def mm(ps, hblk, mlo, half, ki, ci):
                rhs_t = wt[half, ci]
                w = CH[ci]
                if fp8dr:
                    nc.tensor.matmul(
                        ps[:, ci, 0:w],
                        hblk[:, 2 * ki:2 * ki + 2, mlo:mlo + 128],
                        rhs_t[:, 2 * ki:2 * ki + 2, :],
                        start=(ki == 0),
                        stop=(ki == nk - 1),
                        perf_mode=mybir.MatmulPerfMode.DoubleRow,
                    )
                else:
                    nc.tensor.matmul(
                        ps[:, ci, 0:w],
                        hblk[:, ki, mlo:mlo + 128],
                        rhs_t[:, ki, :],
                        start=(ki == 0),
                        stop=(ki == nk - 1),
                    )

            hsums = spool.tile([128, ncols], f32, tag="hsums")

            def act(ps, c0, c1, col):
                # One ACT over banks [c0, c1). Unwritten PSUM cols (the
                # tail of the last bank) read as zero, contributing
                # exp(0)=1 each; host subtracts them.
                esink = kpool.tile([128, nch * 512], bf16, tag="esink")
                nc.scalar.activation(
                    esink[:, c0 * 512:c1 * 512],
                    ps[:, c0:c1, :],
                    AF.Exp,
                    scale=act_scale,
                    accum_out=hsums[:, col:col + 1],
                )

            def step(half, t_i, order, c0=0, col=None):
                if col is None:
                    col = half * tt + t_i
                hblk = ht[t_i // tb]
                mlo = (t_i % tb) * 128
                ps = ppool.tile([128, nch, 512], f32, tag="ps")
                if order == "k":
                    for ki in range(nk):
                        for ci in range(c0, nch):
                            mm(ps, hblk, mlo, half, ki, ci)
                else:
                    for ci in range(c0, nch):
                        for ki in range(nk):
                            mm(ps, hblk, mlo, half, ki, ci)
                act(ps, c0, nch, col)

            # Leading token tiles: one "mini block" first — token tiles
            # 0..nt0-1 against chunk 0 only, one PSUM bank per tile,
            # k-outer so the first 8 matmuls need just the first
            # k-halves of w0_0 + h0 (~0.5MB, the first data to land).
            # Then chunks 1-3 per tile as their DMAs land. Keeps the PE
            # dense through the DMA-paced window so the HAM grant holds.
            w00t = wt[0, 0]

            def mini_mm(ps, bank, t_i, ki):
                mlo = (t_i % tb) * 128
                if fp8dr:
                    nc.tensor.matmul(
                        ps[:, bank, 0:CH[0]],
                        h0[:, 2 * ki:2 * ki + 2, mlo:mlo + 128],
                        w00t[:, 2 * ki:2 * ki + 2, :],
                        start=(ki == 0),
                        stop=(ki == nk - 1),
                        perf_mode=mybir.MatmulPerfMode.DoubleRow,
                    )
                else:
                    nc.tensor.matmul(
                        ps[:, bank, 0:CH[0]],
                        h0[:, ki, mlo:mlo + 128],
                        w00t[:, ki, :],
                        start=(ki == 0),
                        stop=(ki == nk - 1),
                    )

            # Two 2-tile mini allocations so the first pair's ACTs run
            # under the second pair's matmuls (no whole-block ACT chain
            # for the following rest-step to wait on).
            for pair in range(nt0 // 2):
                psm = ppool.tile([128, nch, 512], f32, tag="ps")
                for ki in range(nk):
                    for j in range(2):
                        mini_mm(psm, j, 2 * pair + j, ki)
                for j in range(2):
                    act(psm, j, j + 1, nsteps + 2 + 2 * pair + j)
            for t_i in range(nt0):
                step(0, t_i, "c", c0=1)
            for t_i in range(nt0, tt):
                step(0, t_i, "k")
            nc.sync.dma_start(hsums_d[:, 0:tt], hsums[:, 0:tt])
            nc.sync.dma_start(
                hsums_d[:, nsteps + 2:], hsums[:, nsteps + 2:]
            )
            for t_i in range(tt - 2):
                step(1, t_i, "k")
            nc.sync.dma_start(
                hsums_d[:, tt:nsteps - 2], hsums[:, tt:nsteps - 2]
            )
            # Last two steps split across both PSUM buffers so the ACT
            # pipeline drains with ~1us half-width ACTs and only one
            # such ACT follows the final matmul.
            for t_i in (tt - 2, tt - 1):
                hblk = ht[t_i // tb]
                mlo = (t_i % tb) * 128
                xcol = nsteps + (tt - 1 - t_i)
                psa = ppool.tile([128, nch, 512], f32, tag="ps")
                for ci in range(2):
                    for ki in range(nk):
                        mm(psa, hblk, mlo, 1, ki, ci)
                act(psa, 0, 2, tt + t_i)
                psb = ppool.tile([128, nch, 512], f32, tag="ps")
                for ci in range(2, nch):
                    for ki in range(nk):
                        mm(psb, hblk, mlo, 1, ki, ci)
                act(psb, 2, nch, xcol)
            nc.sync.dma_start(
                hsums_d[:, nsteps - 2:nsteps + 2],
                hsums[:, nsteps - 2:nsteps + 2],
            )


    if do_compile:
        nc.compile()
    return nc


def _get_nc(kt, mode, warm_n=48):
    key = (kt, mode, warm_n)
    if key not in _CACHE:
        _CACHE[key] = _build(kt, mode, warm_n=warm_n)
    return _CACHE[key]


def kernel(hidden_states, head_weight, head_bias, labels, loss_weight):
    from concourse.bass_utils import run_bass_kernel_spmd

    bf16 = ml_dtypes.bfloat16
    fp8 = ml_dtypes.float8_e4m3
    h = np.ascontiguousarray(np.asarray(hidden_states, dtype=np.float32))
    W = np.ascontiguousarray(np.asarray(head_weight, dtype=np.float32))
    b = np.asarray(head_bias, dtype=np.float32)
    lab = np.asarray(labels).astype(np.int64)
    lw = np.asarray(loss_weight, dtype=np.float32)

    use_bias = bool(np.any(b))
    mode = "fp8dr" if (USE_FP8 and not use_bias) else "bf16"
    mdt = fp8 if mode == "fp8dr" else bf16
    wscale = W_SCALE if mode == "fp8dr" else 1.0
    kt = 9 if use_bias else 8
    nc = _get_nc(kt, mode)
    CH = _chunks(CPH)
    nsteps = 2 * TT

    # hT[k, p, t] = h[t, k*128+p]; ht blocks [ntb, 128, kt, TBC].
    hT = np.zeros((kt, 128, T), dtype=np.float32)
    hT[:8] = np.ascontiguousarray(h.T).reshape(8, 128, T)
    if use_bias:
        hT[8, 0, :] = 1.0
    ht_blocks = np.ascontiguousarray(
        hT.reshape(kt, 128, NTB, TBC).transpose(2, 1, 0, 3).astype(mdt)
    )

    Wg = W[lab]                     # [T, D] gathered target rows
    tgt_bias = b[lab]               # [T]

    in_maps = []
    for c in range(NCORES):
        Wc = np.ascontiguousarray(W[c * VSH:(c + 1) * VSH].T) * wscale
        # wT[k, p, v] = Wc.T[k*128+p, v] (scaled)
        wT = np.zeros((kt, 128, VSH), dtype=np.float32)
        wT[:8] = Wc.reshape(8, 128, VSH)
        if use_bias:
            wT[8, 0, :] = b[c * VSH:(c + 1) * VSH]
        m = {}
        off = 0
        for half in range(2):
            for ci, w in enumerate(CH):
                blk = wT[:, :, off:off + w].transpose(1, 0, 2).astype(mdt)
                m[f"w_{half}_{ci}"] = np.ascontiguousarray(blk)
                off += w
        m["ht"] = ht_blocks
        hrow = h[c * TLOC:(c + 1) * TLOC].reshape(JT, 128, D)
        wgrow = Wg[c * TLOC:(c + 1) * TLOC].reshape(JT, 128, D)
        m["hwg"] = np.ascontiguousarray(
            np.stack([hrow, wgrow], axis=2).astype(bf16)
        )
        in_maps.append(m)

    # Tile's scheduler is nondeterministic across builds and has a rare
    # dependency-emission bug: a bad roll yields a NEFF whose outputs are
    # corrupt (dropped accum slots / garbage operands). Validate against
    # hard invariants and an exact host check of the target dots; on
    # failure, rebuild (fresh schedule roll) and rerun.
    pad = len(CH) * 512 - CPH          # zero-region cols per step
    f32 = np.float32

    # Exact host reference for every target dot (same bf16 operands).
    tgt_ref = np.stack([
        (im["hwg"][:, :, 0].astype(f32) * im["hwg"][:, :, 1].astype(f32))
        .sum(axis=2).reshape(TLOC)
        for im in in_maps
    ])                                                      # [8, TLOC]

    # One probe token per token tile, per core: replicates the device's
    # quantized math exactly (same casts) so every accum slot is checked.
    probe_p = (np.arange(TT) * 37) % 128
    probe_tok = np.arange(TT) * 128 + probe_p
    hq = h.astype(mdt).astype(f32)[probe_tok]               # [TT, D]
    if use_bias:
        hq = np.concatenate([hq, np.ones((TT, 1), f32)], axis=1)
    probe_ref = np.empty((NCORES, TT), f32)
    for c in range(NCORES):
        Wc = np.ascontiguousarray(W[c * VSH:(c + 1) * VSH]) * wscale
        Wq = Wc.astype(mdt).astype(f32)                     # [VSH, D]
        if use_bias:
            bq = b[c * VSH:(c + 1) * VSH].astype(mdt).astype(f32)
            Wq = np.concatenate([Wq, bq[:, None]], axis=1)
        lg = (hq @ Wq.T) / wscale
        probe_ref[c] = np.exp(lg).sum(axis=1)

    for attempt in range(4):
        res = run_bass_kernel_spmd(nc, in_maps, core_ids=list(range(NCORES)))

        # hsums[c][p, half*TT+t] are partial sums of exp(logit) over half
        # of core c's vocab shard for token t*128+p (+pad zero-cols).
        # Extra cols: [nsteps]/[nsteps+1] = trailing banks of the last
        # two steps, [nsteps+2+t] = chunk-0 minis of the leading token
        # tiles; fold them in.
        Sfull = np.stack([r["hsums"] for r in res.results])  # [8,128,ncols]
        Sraw = np.ascontiguousarray(Sfull[:, :, :nsteps])
        Sraw[:, :, nsteps - 1] += Sfull[:, :, nsteps]
        Sraw[:, :, nsteps - 2] += Sfull[:, :, nsteps + 1]
        Sraw[:, :, :NT0] += Sfull[:, :, nsteps + 2:]
        G = np.stack([r["tgt"] for r in res.results])       # [8, 128, JT]
        err_state = np.seterr(over="ignore", invalid="ignore")
        dev_probe = (
            Sraw[:, probe_p, np.arange(TT)]
            + Sraw[:, probe_p, TT + np.arange(TT)]
            - 2.0 * pad
        )                                                   # [8, TT]
        g_dev = G.transpose(0, 2, 1).reshape(NCORES, TLOC)
        ok = (
            np.isfinite(Sfull).all()
            and np.isfinite(G).all()
            and (Sraw > pad).all()
            and np.allclose(g_dev, tgt_ref, rtol=2e-2, atol=1e-2)
            and np.allclose(dev_probe, probe_ref, rtol=5e-2, atol=1.0)
        )
        np.seterr(**err_state)
        if ok:
            break
        nc = _get_nc(kt, mode, warm_n=48 + 2 * (attempt + 1))
    if not ok:
        # Every compile rolled a bad schedule: compute on host (slow but
        # exact) rather than return a corrupt result.
        logits = h @ W.T + b
        mx = logits.max(axis=1, keepdims=True)
        logz = np.log(
            np.exp((logits - mx).astype(np.float64)).sum(axis=1)
        ) + mx[:, 0]
        nll = logz - logits[np.arange(T), lab]
        lw64 = lw.astype(np.float64)
        return np.float32((lw64 * nll).sum() / lw64.sum())

    S = Sraw.reshape(NCORES, 128, 2, TT).sum(axis=2)        # [8,128,TT]
    sumexp = S.transpose(0, 2, 1).reshape(NCORES, T).astype(np.float64)
    sumexp -= 2.0 * pad
    logz = np.log(sumexp.sum(axis=0))                       # [T]

    tgt = G.transpose(0, 2, 1).reshape(T) + tgt_bias        # [T]

    nll = logz - tgt
    lw64 = lw.astype(np.float64)
    loss = (lw64 * nll).sum() / lw64.sum()
    return np.float32(loss)
